# revision 1
# baseline (speedup 1.0000x reference)
"""Trainium2 Bass kernel for the EnhancedMathematicalReasoning MoE-routing module.

Computation (per token t, hidden dim H=2048, E=8 experts, dense routing):
    a1     = gelu(h @ Wd1 + bd1)
    logits = a1 @ Wd2 + bd2
    op_w   = softmax(logits)
    comb   = sum_e op_w[:, e] * (h @ We[e] + be[e])
    out    = (gelu(comb @ Wi1 + bi1) @ Wi2 + bi2) * mask

Sharding: data-parallel over the 8192 tokens -> 1024 tokens per NeuronCore,
weights replicated, no collectives.

Per-core layout strategy (P=128):
  - h is PE-transposed once to hT [H, T] so H sits on partitions for every
    GEMM contraction.  All big GEMMs run in float32r (TF32 datapath - full PE
    rate at moving free dim >= 256, ~1e-4 relative accuracy).
  - GEMM1/GEMM3/GEMM4 produce [H_out, T] with the weight m-chunk stationary
    and the resident activation as a 512-wide moving operand.
  - Expert GEMMs run in [T, H_out] orientation (hT slices stationary, We
    chunks moving at N=256) so op_w[t, e] is a per-partition scalar and the
    weighted combine is one fused DVE op per psum eviction:
        comb = psum * op_w[:, e] + comb
  - All PE transposes are batched 4-to-a-PSUM-bank with a single batched
    eviction that alternates between the Scalar and Vector engines, keeping
    the PE fed during transpose phases.
  - comb -> combT transposes are interleaved into the last expert's tail;
    the final out-transpose + mask + store are interleaved into GEMM4.
"""

import numpy as np
from contextlib import ExitStack

import concourse.bass as bass
import concourse.tile as tile
from concourse import bacc, mybir
from concourse.bass_utils import run_bass_kernel_spmd
from concourse.masks import make_identity

F32 = mybir.dt.float32
F32R = mybir.dt.float32r
AF = mybir.ActivationFunctionType
ALU = mybir.AluOpType
AX = mybir.AxisListType

P = 128
N_CORES = 8

B, S, H_FULL, E_FULL = 4, 2048, 2048, 8


def build_nc(T, H, E, act=AF.Gelu, include_be=False):
    """Build + compile the single-core program (same program runs SPMD on all
    cores). T: tokens per core. Requires T % 512 == 0, H % 512 == 0."""
    assert T % 512 == 0 and H % 512 == 0 and E <= P
    KT = H // P
    TT = T // P
    TB = T // 512
    NB = H // 256
    MT = H // P

    nc = bacc.Bacc("TRN2", target_bir_lowering=False, debug=False)

    h_d = nc.dram_tensor("h", [T, H], F32R, kind="ExternalInput").ap()
    msk_d = nc.dram_tensor("mask", [T], F32, kind="ExternalInput").ap()
    wd1_d = nc.dram_tensor("wd1", [H, H], F32R, kind="ExternalInput").ap()
    bd1_d = nc.dram_tensor("bd1", [H], F32, kind="ExternalInput").ap()
    wd2_d = nc.dram_tensor("wd2", [H, E], F32R, kind="ExternalInput").ap()
    bd2_d = nc.dram_tensor("bd2", [E], F32, kind="ExternalInput").ap()
    we_d = nc.dram_tensor("we", [E, H, H], F32R, kind="ExternalInput").ap()
    be_d = nc.dram_tensor("be", [E, H], F32R, kind="ExternalInput").ap()
    wi1_d = nc.dram_tensor("wi1", [H, H], F32R, kind="ExternalInput").ap()
    bi1_d = nc.dram_tensor("bi1", [H], F32, kind="ExternalInput").ap()
    wi2_d = nc.dram_tensor("wi2", [H, H], F32R, kind="ExternalInput").ap()
    bi2_d = nc.dram_tensor("bi2", [H], F32, kind="ExternalInput").ap()
    out_d = nc.dram_tensor("out", [T, H], F32, kind="ExternalOutput").ap()

    wd1_re = wd1_d.rearrange("(k p) n -> p k n", p=P)
    wi1_re = wi1_d.rearrange("(k p) n -> p k n", p=P)
    wi2_re = wi2_d.rearrange("(k p) n -> p k n", p=P)

    with tile.TileContext(nc) as tc:
        with ExitStack() as ctx:
            const = ctx.enter_context(tc.tile_pool(name="const", bufs=1))
            bigp = ctx.enter_context(tc.tile_pool(name="bigp", bufs=1))
            wep = ctx.enter_context(tc.tile_pool(name="wep", bufs=2))
            scr = ctx.enter_context(tc.tile_pool(name="scr", bufs=2))
            osm = ctx.enter_context(tc.tile_pool(name="osm", bufs=2))
            opb = ctx.enter_context(tc.tile_pool(name="opb", bufs=3))
            smp = ctx.enter_context(tc.tile_pool(name="smp", bufs=4))
            pp = ctx.enter_context(tc.tile_pool(name="pp", bufs=2, space="PSUM"))

            # ---- constants (engine-generated; no DMA) ----
            ident = const.tile([P, P], F32, name="ident")
            make_identity(nc, ident)
            ones1 = const.tile([1, P], F32, name="ones1")
            nc.vector.memset(ones1, 1.0)
            identR = const.tile([P, P], F32R, name="identR")
            nc.scalar.copy(identR, ident)
            # sel8[e', e*128+p] = (e' == e): K=8 selector used to broadcast
            # op_w rows across all 128 partitions via a tiny matmul.
            sel8f = const.tile([E, E, P], F32, name="sel8f")
            nc.gpsimd.memset(sel8f, 0.0)
            nc.gpsimd.affine_select(
                out=sel8f, in_=sel8f, compare_op=ALU.not_equal, fill=1.0,
                base=0, pattern=[[-1, E], [0, P]], channel_multiplier=1)
            sel8 = const.tile([E, E * P], F32R, name="sel8")
            nc.scalar.copy(sel8, sel8f.rearrange("e a p -> e (a p)"))
            opwT = const.tile([E, T], F32R, name="opwT")
            opw = const.tile([P, TT, E], F32, name="opw")
            lacc = const.tile([P, TT, E], F32, name="lacc")

            # Batched PE transpose: up to 4 [128,128] tiles share one PSUM bank
            # (one accumulation group, disjoint column writes), then a single
            # batched eviction on an alternating engine keeps ACT/DVE balanced.
            ecnt = [0]

            def tbatch(srcs, out3, scale=None, dma=None):
                n = len(srcs)
                trp = pp.tile([P, 4, P], F32R, tag="tr", bufs=3, name="trb")
                for i, s in enumerate(srcs):
                    nc.tensor.matmul(trp[:, i, :], s, identR, is_transpose=True,
                                     start=(i == 0), stop=(i == n - 1))
                src = trp[:, :n, :]
                ecnt[0] += 1
                if dma is not None:
                    ot = osm.tile([P, 4, P], F32, tag="os", name="ot")
                    dst = ot[:, :n, :]
                else:
                    dst = out3
                if scale is None:
                    if ecnt[0] % 2 == 0:
                        nc.scalar.copy(dst, src)
                    else:
                        nc.vector.tensor_copy(dst, src)
                else:
                    if ecnt[0] % 2 == 0:
                        nc.scalar.activation(dst, src, AF.Copy, scale=scale)
                    else:
                        nc.vector.tensor_scalar_mul(dst, src, scale)
                if dma is not None:
                    nc.sync.dma_start(dma, dst)

            # ---- stage A: load h, transpose to hT [H, T] ----
            hT = bigp.tile([P, KT, T], F32R, tag="A", name="hT")
            w1_0 = None
            ttorder = list(range(TT))
            ttorder.insert(min(4, TT), -1)
            for tt in ttorder:
                if tt == -1:
                    # GEMM1's first weight chunk: issued here so its DMA is not
                    # queued behind the tt4..7 h-loads (12us PE gap otherwise).
                    w1_0 = wep.tile([P, KT, 256], F32R, tag="we", name="wd1m_0")
                    nc.sync.dma_start(w1_0, wd1_re[:, :, 0:256])
                    continue
                for half in range(2):
                    hw = min(1024, H - half * 1024)
                    if hw <= 0:
                        continue
                    hl = scr.tile([P, 1024], F32R, tag="s", bufs=3, name=f"hl_{tt}_{half}")
                    nc.sync.dma_start(
                        hl[:, :hw],
                        h_d[tt * P:(tt + 1) * P, half * 1024:half * 1024 + hw])
                    for kg in range(hw // P // 4):
                        k0 = half * 8 + kg * 4
                        tbatch(
                            [hl[:, (kg * 4 + j) * P:(kg * 4 + j + 1) * P]
                             for j in range(4)],
                            hT[:, k0:k0 + 4, tt * P:(tt + 1) * P])

            # ---- constant DMA loads (emitted after stage A so the h
            # loads win the DMA queues at kernel start) ----
            wd2_t = const.tile([P, KT, E], F32R, name="wd2_t")
            nc.sync.dma_start(wd2_t, wd2_d.rearrange("(k p) e -> p k e", p=P))
            bd1_t = const.tile([P, KT], F32, name="bd1_t")
            nc.sync.dma_start(bd1_t, bd1_d.rearrange("(k p) -> p k", p=P))
            bi1_t = const.tile([P, KT], F32, name="bi1_t")
            nc.sync.dma_start(bi1_t, bi1_d.rearrange("(k p) -> p k", p=P))
            bi2_t = const.tile([P, KT], F32, name="bi2_t")
            nc.sync.dma_start(bi2_t, bi2_d.rearrange("(k p) -> p k", p=P))
            bd2_t = const.tile([1, E], F32, name="bd2_t")
            nc.sync.dma_start(bd2_t, bd2_d.unsqueeze(0))
            mask_t = const.tile([P, TT], F32, name="mask_t")
            nc.sync.dma_start(mask_t, msk_d.rearrange("(t p) -> p t", p=P))
            if include_be:
                be_t = const.tile([E, H], F32R, name="be_t")
                nc.sync.dma_start(be_t, be_d)

            # ---- stage B: a1T = act(Wd1.T @ hT + bd1) + fused logits GEMM ----
            for mg in range(MT // 2):
                if mg == 0 and w1_0 is not None:
                    w1 = w1_0
                else:
                    w1 = wep.tile([P, KT, 256], F32R, tag="we", name=f"wd1m_{mg}")
                    nc.sync.dma_start(w1, wd1_re[:, :, mg * 256:(mg + 1) * 256])
                for mi in range(2):
                    m = 2 * mg + mi
                    a1 = scr.tile([P, T], F32R, tag="s", bufs=3, name=f"a1_{m}")
                    for tb in range(TB):
                        ps = pp.tile([P, 512], F32, tag="mm", bufs=3, name="ps_g1")
                        for k in range(KT):
                            nc.tensor.matmul(ps, w1[:, k, mi * P:(mi + 1) * P],
                                             hT[:, k, tb * 512:(tb + 1) * 512],
                                             start=(k == 0), stop=(k == KT - 1))
                        nc.scalar.activation(a1[:, tb * 512:(tb + 1) * 512], ps,
                                             act, bias=bd1_t[:, m:m + 1])
                    for tt in range(TT):
                        lg = pp.tile([P, E], F32, tag="lgt", bufs=1, name="lg")
                        nc.tensor.matmul(lg, a1[:, tt * P:(tt + 1) * P],
                                         wd2_t[:, m, :], start=True, stop=(m > 0))
                        if m == 0:
                            nc.tensor.matmul(lg, ones1, bd2_t, start=False,
                                             stop=True)
                            nc.vector.tensor_copy(lacc[:, tt, :], lg)
                        else:
                            nc.vector.tensor_add(lacc[:, tt, :], lacc[:, tt, :], lg)

            # ---- softmax over E ----
            for tt in range(TT):
                nmax = smp.tile([P, 1], F32, tag="sm1", bufs=6, name="nmax")
                nc.vector.reduce_max(nmax, lacc[:, tt, :], AX.X, negate=True)
                et = smp.tile([P, E], F32, tag="sme", bufs=2, name="et")
                nc.scalar.activation(et, lacc[:, tt, :], AF.Exp, bias=nmax,
                                     scale=1.0)
                ssum = smp.tile([P, 1], F32, tag="sm1", bufs=6, name="ssum")
                nc.vector.reduce_sum(ssum, et, AX.X)
                rin = smp.tile([P, 1], F32, tag="sm1", bufs=6, name="rin")
                nc.vector.reciprocal(rin, ssum)
                nc.vector.tensor_scalar_mul(opw[:, tt, :], et, rin)
            # Hoisted: the first expert chunk's psum fills depend only on hT
            # and We[0], so they keep the PE busy while the serial softmax ->
            # op_w-transpose -> broadcast chain resolves on DVE/ACT.
            wet00 = wep.tile([P, KT, 256], F32R, tag="we", name="we_0_0")
            nc.sync.dma_start(
                wet00,
                we_d[0].rearrange("(k p) n -> p k n", p=P)[:, :, 0:256])
            hoist_ps = []
            for tb in range(TB):
                ps = pp.tile([P, 512], F32, tag="mm", bufs=3, name="eps_h")
                for k in range(KT):
                    nc.tensor.matmul(ps, wet00[:, k, 0:P],
                                     hT[:, k, tb * 512:(tb + 1) * 512],
                                     start=(k == 0), stop=(k == KT - 1))
                hoist_ps.append(ps)
            for tt in range(TT):
                trp = pp.tile([P, 4, P], F32, tag="tr", bufs=3, name="trp_ow")
                nc.tensor.matmul(trp[:E, 0, :], opw[:, tt, :], ident,
                                 is_transpose=True, start=True, stop=True)
                nc.scalar.copy(opwT[:, tt * P:(tt + 1) * P], trp[:E, 0, :])

            # ---- stage C: expert GEMMs in [H_out, T] orientation.
            # Stationary = We m-chunk, moving = resident hT at N=512 (full
            # fp32r rate, ~227ns per 512-row MM).  op_w[t, e] is broadcast
            # across partitions as opb = sel8[:, e].T @ opwT (a K=8 matmul),
            # and the weighted combine accumulates straight into combT [H, T]:
            #     combT[m, t] += opb[t] * psum[m, t]
            # eliminating the [T,H] comb buffer and its 128 PE transposes.
            arena = bigp.tile([P, KT, TT, P], F32, tag="B", name="arena")
            arenaR = arena.bitcast(F32R)

            if include_be:
                # init combT with the op_w-weighted bias term:
                #   combT[m*128+p, t] = sum_e op_w[t, e] * be[e, m*128+p]
                for m in range(MT):
                    for tb in range(TB):
                        bps = pp.tile([P, 512], F32, tag="mm", bufs=3, name="bps")
                        nc.tensor.matmul(bps, be_t[:, m * P:(m + 1) * P],
                                         opwT[:, tb * 512:(tb + 1) * 512],
                                         start=True, stop=True)
                        nc.scalar.copy(
                            arenaR[:, m, tb * 4:(tb + 1) * 4, :],
                            bps.rearrange("p (n c) -> p n c", c=P))

            obs = {}
            for e in range(E):
                we_re = we_d[e].rearrange("(k p) n -> p k n", p=P)
                for tb in range(TB):
                    bps = pp.tile([P, 512], F32, tag="mm", bufs=3, name="bps")
                    nc.tensor.matmul(bps, sel8[:, e * P:(e + 1) * P],
                                     opwT[:, tb * 512:(tb + 1) * 512],
                                     start=True, stop=True)
                    ob = opb.tile([P, 512], F32, tag="ob", bufs=3,
                                  name=f"ob_{e}_{tb}")
                    nc.scalar.copy(ob, bps)
                    obs[tb] = ob
                for mg in range(MT // 2):
                    if e == 0 and mg == 0:
                        wet = wet00
                    else:
                        wet = wep.tile([P, KT, 256], F32R, tag="we",
                                       name=f"we_{e}_{mg}")
                        nc.sync.dma_start(wet,
                                          we_re[:, :, mg * 256:(mg + 1) * 256])
                    for mi in range(2):
                        m = 2 * mg + mi
                        for tb in range(TB):
                            if e == 0 and mg == 0 and mi == 0:
                                ps = hoist_ps[tb]
                            else:
                                ps = pp.tile([P, 512], F32, tag="mm", bufs=3,
                                             name="eps")
                                for k in range(KT):
                                    nc.tensor.matmul(
                                        ps, wet[:, k, mi * P:(mi + 1) * P],
                                        hT[:, k, tb * 512:(tb + 1) * 512],
                                        start=(k == 0), stop=(k == KT - 1))
                            wsl = arenaR[:, m, tb * 4:(tb + 1) * 4, :]
                            rsl = arena[:, m, tb * 4:(tb + 1) * 4, :]
                            ob3 = obs[tb].rearrange("p (n c) -> p n c", c=P)
                            ps3 = ps.rearrange("p (n c) -> p n c", c=P)
                            if e == 0 and not include_be:
                                nc.vector.tensor_tensor(wsl, ps3, ob3,
                                                        op=ALU.mult)
                            else:
                                tmp = scr.tile([P, 512], F32, tag="s",
                                               bufs=3, name="tmp")
                                tmp3 = tmp.rearrange("p (n c) -> p n c", c=P)
                                nc.vector.tensor_tensor(tmp3, ps3, ob3,
                                                        op=ALU.mult)
                                nc.vector.tensor_tensor(wsl, rsl, tmp3,
                                                        op=ALU.add)

            # ---- stage E: a2T = act(Wi1.T @ combT + bi1) ----
            a2T = bigp.tile([P, KT, T], F32R, tag="A", name="a2T")
            for mg in range(MT // 2):
                w3 = wep.tile([P, KT, 256], F32R, tag="we", name=f"wi1m_{mg}")
                nc.sync.dma_start(w3, wi1_re[:, :, mg * 256:(mg + 1) * 256])
                for mi in range(2):
                    m = 2 * mg + mi
                    for tb in range(TB):
                        ps = pp.tile([P, 512], F32, tag="mm", bufs=3, name="ps_g3")
                        for k in range(KT):
                            nc.tensor.matmul(ps, w3[:, k, mi * P:(mi + 1) * P],
                                             arenaR[:, k, tb * 4:(tb + 1) * 4, :],
                                             start=(k == 0), stop=(k == KT - 1))
                        nc.scalar.activation(a2T[:, m, tb * 512:(tb + 1) * 512],
                                             ps, act, bias=bi1_t[:, m:m + 1])

            # ---- stage F: outT = Wi2.T @ a2T + bi2, with the out-transpose,
            #      mask and store interleaved every 4 m-tiles ----
            outT = bigp.tile([P, KT, T], F32R, tag="B", name="outT")
            for mg in range(MT // 2):
                w4 = wep.tile([P, KT, 256], F32R, tag="we", name=f"wi2m_{mg}")
                nc.sync.dma_start(w4, wi2_re[:, :, mg * 256:(mg + 1) * 256])
                for mi in range(2):
                    m = 2 * mg + mi
                    for tb in range(TB):
                        ps = pp.tile([P, 512], F32, tag="mm", bufs=3, name="ps_g4")
                        for k in range(KT):
                            nc.tensor.matmul(ps, w4[:, k, mi * P:(mi + 1) * P],
                                             a2T[:, k, tb * 512:(tb + 1) * 512],
                                             start=(k == 0), stop=(k == KT - 1))
                        nc.scalar.activation(outT[:, m, tb * 512:(tb + 1) * 512],
                                             ps, AF.Identity,
                                             bias=bi2_t[:, m:m + 1])
                    if m % 4 == 3:
                        for tt in range(TT):
                            tbatch(
                                [outT[:, m - 3 + j, tt * P:(tt + 1) * P]
                                 for j in range(4)],
                                None,
                                scale=mask_t[:, tt:tt + 1],
                                dma=out_d[tt * P:(tt + 1) * P,
                                          (m - 3) * P:(m + 1) * P].rearrange(
                                              "t (n c) -> t n c", c=P))

    nc.compile()
    return nc


_CACHED = {}


def _get_nc(T, H, E, include_be):
    key = (T, H, E, include_be)
    if key not in _CACHED:
        _CACHED[key] = build_nc(T, H, E, act=AF.Gelu, include_be=include_be)
    return _CACHED[key]


def kernel(hidden_states, attention_mask, Wd1, bd1, Wd2, bd2, We, be, Wi1, bi1,
           Wi2, bi2, _trace=False):
    f32 = lambda x: np.ascontiguousarray(np.asarray(x, dtype=np.float32))
    h = f32(hidden_states)
    mask = f32(attention_mask)
    Wd1, bd1, Wd2, bd2 = f32(Wd1), f32(bd1), f32(Wd2), f32(bd2)
    We, be, Wi1, bi1, Wi2, bi2 = f32(We), f32(be), f32(Wi1), f32(bi1), f32(Wi2), f32(bi2)

    Bv, Sv, Hv = h.shape
    Ev = Wd2.shape[1]
    TOK = Bv * Sv
    T = TOK // N_CORES
    include_be = bool(np.any(be))

    nc = _get_nc(T, Hv, Ev, include_be)

    hf = h.reshape(TOK, Hv)
    mf = mask.reshape(TOK)
    weights = dict(wd1=Wd1, bd1=bd1, wd2=Wd2, bd2=bd2, we=We, be=be,
                   wi1=Wi1, bi1=bi1, wi2=Wi2, bi2=bi2)
    in_maps = []
    for c in range(N_CORES):
        m = dict(weights)
        m["h"] = np.ascontiguousarray(hf[c * T:(c + 1) * T])
        m["mask"] = np.ascontiguousarray(mf[c * T:(c + 1) * T])
        in_maps.append(m)

    # The first execution of a freshly-loaded NEFF occasionally trips a
    # transient NRT_EXEC_UNIT_UNRECOVERABLE on the axon worker; a retry after a
    # short pause has always succeeded, so tolerate a couple of those.
    last_exc = None
    for attempt in range(3):
        try:
            res = run_bass_kernel_spmd(nc, in_maps,
                                       core_ids=list(range(N_CORES)),
                                       trace=_trace)
            break
        except Exception as e:  # noqa: BLE001 - jax.errors.JaxRuntimeError
            last_exc = e
            if "UNAVAILABLE" not in str(e) and "unrecoverable" not in str(e):
                raise
            import time as _time
            _time.sleep(5 * (attempt + 1))
    else:
        raise last_exc
    out = np.concatenate([res.results[c]["out"] for c in range(N_CORES)], axis=0)
    out = out.reshape(Bv, Sv, Hv).astype(np.float32)
    if _trace:
        kernel._last_results = res
    return out



# revision 2
# speedup vs baseline: 1.1951x; 1.1951x over previous
"""Trainium2 Bass kernel for the EnhancedMathematicalReasoning MoE-routing module.

Computation (per token t, hidden dim H=2048, E=8 experts, dense routing):
    a1     = gelu(h @ Wd1 + bd1)
    logits = a1 @ Wd2 + bd2
    op_w   = softmax(logits)
    comb   = sum_e op_w[:, e] * (h @ We[e] + be[e])
    out    = (gelu(comb @ Wi1 + bi1) @ Wi2 + bi2) * mask

Sharding: data-parallel over the 8192 tokens -> 1024 tokens per NeuronCore,
weights replicated, no collectives.

v2 layout strategy (P=128):
  - h is transposed ON THE HOST to hT [H, T] so the kernel loads it directly
    with H on partitions (no on-device PE transposes; kills the old stage A).
  - The output stays in [H, T] orientation and is un-transposed on the host,
    killing the old 128 out-transposes and the transpose+store tail.
  - All big GEMMs produce [H_out, T] with the weight m-chunk stationary and
    the resident activation as a 512-wide moving operand.
  - Expert GEMMs accumulate over k in PSUM; op_w[t, e] is broadcast across
    partitions via a K=8 selector matmul and the weighted combine is fused
    DVE ops per psum eviction into an fp32 arena (= combT).
  - The mask multiply is skipped entirely when the host sees an all-ones
    attention_mask (the general path broadcasts mask across partitions and
    applies one DVE multiply per output eviction).
"""

import numpy as np
from contextlib import ExitStack

import concourse.bass as bass
import concourse.tile as tile
from concourse import bacc, mybir
from concourse.bass_utils import run_bass_kernel_spmd
from concourse.masks import make_identity

F32 = mybir.dt.float32
F32R = mybir.dt.float32r
BF16 = mybir.dt.bfloat16
AF = mybir.ActivationFunctionType
ALU = mybir.AluOpType
AX = mybir.AxisListType

P = 128
N_CORES = 8

B, S, H_FULL, E_FULL = 4, 2048, 2048, 8

# operand dtype for the big GEMMs: F32R (tf32-like) or BF16.
# bf16 wins on HW: the fp32r moving operand pays ~+14ns per 512-col matmul
# (227 vs 216 ns measured), and bf16 halves DMA traffic + SBUF footprint.
# Accuracy: all-bf16 operands with fp32 PSUM accumulation measure 4.6e-3
# rel-l2 against the fp32 reference (tolerance 2e-2).
GEMM_DT = BF16


def build_nc(T, H, E, act=AF.Gelu, include_be=False, apply_mask=True,
             dt=GEMM_DT):
    """Build + compile the single-core program (same program runs SPMD on all
    cores). T: tokens per core. Requires T % 512 == 0, H % 512 == 0."""
    assert T % 512 == 0 and H % 512 == 0 and E <= P
    KT = H // P
    TT = T // P
    TB = T // 512
    MT = H // P

    nc = bacc.Bacc("TRN2", target_bir_lowering=False, debug=False)

    # packed on host: ht[tb, kg, p, (4k 512t)] so every DMA line is
    # partition-contiguous (4 KB+) instead of 0.5-1 KB row fragments
    ht_d = nc.dram_tensor("ht", [T // 512, KT // 4, P, 4 * 512], dt,
                          kind="ExternalInput").ap()
    msk_d = nc.dram_tensor("mask", [T], F32R, kind="ExternalInput").ap()
    wd1_d = nc.dram_tensor("wd1", [H // 256, P, KT * 256], dt,
                           kind="ExternalInput").ap()
    bd1_d = nc.dram_tensor("bd1", [H], F32, kind="ExternalInput").ap()
    wd2_d = nc.dram_tensor("wd2", [H, E], dt, kind="ExternalInput").ap()
    bd2_d = nc.dram_tensor("bd2", [E], F32, kind="ExternalInput").ap()
    we_d = nc.dram_tensor("we", [E, H // 256, P, KT * 256], dt,
                          kind="ExternalInput").ap()
    be_d = nc.dram_tensor("be", [E, H], F32R, kind="ExternalInput").ap()
    wi1_d = nc.dram_tensor("wi1", [H // 256, P, KT * 256], dt,
                           kind="ExternalInput").ap()
    bi1_d = nc.dram_tensor("bi1", [H], F32, kind="ExternalInput").ap()
    wi2_d = nc.dram_tensor("wi2", [H // 256, P, KT * 256], dt,
                           kind="ExternalInput").ap()
    bi2_d = nc.dram_tensor("bi2", [H], F32, kind="ExternalInput").ap()
    out_d = nc.dram_tensor("out", [H, T], F32, kind="ExternalOutput").ap()


    with tile.TileContext(nc) as tc:
        with ExitStack() as ctx:
            const = ctx.enter_context(tc.tile_pool(name="const", bufs=1))
            bigp = ctx.enter_context(tc.tile_pool(name="bigp", bufs=1))
            wep = ctx.enter_context(tc.tile_pool(name="wep", bufs=3))
            scr = ctx.enter_context(tc.tile_pool(name="scr", bufs=2))
            osm = ctx.enter_context(tc.tile_pool(name="osm", bufs=3))
            smp = ctx.enter_context(tc.tile_pool(name="smp", bufs=4))
            pp = ctx.enter_context(tc.tile_pool(name="pp", bufs=2, space="PSUM"))

            # ---- first loads: hT + GEMM1's first weight chunk win the DMA
            # queues at kernel start.  Split into k-quarters so GEMM1's first
            # psum group starts as soon as the first ~0.5 MB lands. ----
            hT = bigp.tile([P, KT, T], dt, tag="A", name="hT")
            w1_0 = wep.tile([P, KT, 256], dt, tag="we", name="wd1m_0")
            for kg in range(KT // 4):
                ks = slice(kg * 4, (kg + 1) * 4)
                nc.sync.dma_start(
                    hT[:, ks, 0:512],
                    ht_d[0, kg].rearrange("p (k t) -> p k t", k=4))
                nc.sync.dma_start(
                    w1_0[:, ks, :],
                    wd1_d[0, :, kg * 1024:(kg + 1) * 1024].rearrange(
                        "p (k n) -> p k n", k=4))
            for tb in range(1, TB):
                for kg in range(KT // 4):
                    ks = slice(kg * 4, (kg + 1) * 4)
                    nc.sync.dma_start(
                        hT[:, ks, tb * 512:(tb + 1) * 512],
                        ht_d[tb, kg].rearrange("p (k t) -> p k t", k=4))

            # ---- constants (engine-generated; no DMA) ----
            ident = const.tile([P, P], F32, name="ident")
            make_identity(nc, ident)
            ones1 = const.tile([1, P], F32, name="ones1")
            nc.vector.memset(ones1, 1.0)
            # sel8[e', e*128+p] = (e' == e): K=8 selector used to broadcast
            # op_w rows across all 128 partitions via a tiny matmul.
            sel8f = const.tile([E, E, P], F32, name="sel8f")
            nc.gpsimd.memset(sel8f, 0.0)
            nc.gpsimd.affine_select(
                out=sel8f, in_=sel8f, compare_op=ALU.not_equal, fill=1.0,
                base=0, pattern=[[-1, E], [0, P]], channel_multiplier=1)
            sel8 = const.tile([E, E * P], dt, name="sel8")
            nc.scalar.copy(sel8, sel8f.rearrange("e a p -> e (a p)"))
            opwT = const.tile([E, T], dt, name="opwT")
            opw = const.tile([P, TT, E], F32, name="opw")

            # ---- constant DMA loads ----
            wd2_t = const.tile([P, KT, E], dt, name="wd2_t")
            nc.sync.dma_start(wd2_t, wd2_d.rearrange("(k p) e -> p k e", p=P))
            bd1_t = const.tile([P, KT], F32, name="bd1_t")
            nc.sync.dma_start(bd1_t, bd1_d.rearrange("(k p) -> p k", p=P))
            bi1_t = const.tile([P, KT], F32, name="bi1_t")
            nc.sync.dma_start(bi1_t, bi1_d.rearrange("(k p) -> p k", p=P))
            bi2_t = const.tile([P, KT], F32, name="bi2_t")
            nc.sync.dma_start(bi2_t, bi2_d.rearrange("(k p) -> p k", p=P))
            bd2_t = const.tile([1, E], F32, name="bd2_t")
            nc.sync.dma_start(bd2_t, bd2_d.unsqueeze(0))
            if apply_mask:
                mrow = const.tile([1, T], F32R, name="mrow")
                nc.sync.dma_start(mrow, msk_d.unsqueeze(0))
                onesP = const.tile([1, P], F32R, name="onesP")
                nc.vector.memset(onesP, 1.0)
                maskb = const.tile([P, TT, P], F32, name="maskb")
                for tb in range(TB):
                    mps = pp.tile([P, 512], F32, tag="mm", bufs=3, name="mps")
                    nc.tensor.matmul(mps, onesP,
                                     mrow[:, tb * 512:(tb + 1) * 512],
                                     start=True, stop=True)
                    nc.vector.tensor_copy(
                        maskb[:, tb * 4:(tb + 1) * 4, :],
                        mps.rearrange("p (n c) -> p n c", c=P))
            if include_be:
                be_r = const.tile([E, H], F32R, name="be_r")
                nc.sync.dma_start(be_r, be_d)
                be_t = const.tile([E, H], dt, name="be_t")
                nc.scalar.copy(be_t, be_r)

            # ---- stage B: a1 = act(Wd1.T @ hT + bd1) + fused logits GEMM.
            # The logits accumulate across all m directly in ONE psum bank
            # (disjoint 8-wide regions, one accumulation group per token
            # tile), so no DVE adds pile up ahead of the softmax. ----
            lgp = pp.tile([P, TT, E], F32, tag="lgt", bufs=1, name="lgp")
            for mg in range(MT // 2):
                if mg == 0:
                    w1 = w1_0
                else:
                    w1 = wep.tile([P, KT, 256], dt, tag="we", name=f"wd1m_{mg}")
                    nc.sync.dma_start(
                        w1, wd1_d[mg].rearrange("p (k n) -> p k n", k=KT))
                for mi in range(2):
                    m = 2 * mg + mi
                    a1 = scr.tile([P, T], dt, tag="s", bufs=3, name=f"a1_{m}")
                    for tb in range(TB):
                        ps = pp.tile([P, 512], F32, tag="mm", bufs=3, name="ps_g1")
                        for k in range(KT):
                            nc.tensor.matmul(ps, w1[:, k, mi * P:(mi + 1) * P],
                                             hT[:, k, tb * 512:(tb + 1) * 512],
                                             start=(k == 0), stop=(k == KT - 1))
                        nc.scalar.activation(a1[:, tb * 512:(tb + 1) * 512], ps,
                                             act, bias=bd1_t[:, m:m + 1])
                    for tt in range(TT):
                        # single accumulation group for the whole bank:
                        # start=True clears the bank-wide has_written bits, so
                        # it must fire exactly once (first write); every other
                        # region's first write self-initializes element-wise.
                        nc.tensor.matmul(lgp[:, tt, :],
                                         a1[:, tt * P:(tt + 1) * P],
                                         wd2_t[:, m, :],
                                         start=(m == 0 and tt == 0),
                                         stop=False, skip_group_check=True)
            for tt in range(TT):
                nc.tensor.matmul(lgp[:, tt, :], ones1, bd2_t, start=False,
                                 stop=(tt == TT - 1), skip_group_check=True)

            # ---- softmax over E (reads the logits psum directly) ----
            for tt in range(TT):
                nmax = smp.tile([P, 1], F32, tag="sm1", bufs=6, name="nmax")
                nc.vector.reduce_max(nmax, lgp[:, tt, :], AX.X, negate=True)
                et = smp.tile([P, E], F32, tag="sme", bufs=2, name="et")
                nc.scalar.activation(et, lgp[:, tt, :], AF.Exp, bias=nmax,
                                     scale=1.0)
                ssum = smp.tile([P, 1], F32, tag="sm1", bufs=6, name="ssum")
                nc.vector.reduce_sum(ssum, et, AX.X)
                rin = smp.tile([P, 1], F32, tag="sm1", bufs=6, name="rin")
                nc.vector.reciprocal(rin, ssum)
                nc.vector.tensor_scalar_mul(opw[:, tt, :], et, rin)
            # Hoisted PE work that does NOT depend on the softmax: the first
            # expert chunk's psum groups (2 held "hst" banks) plus up to three
            # more groups parked on the cycling "mm" banks give the PE ~20us
            # of cover while the serial softmax -> op_w-transpose -> broadcast
            # chain resolves on DVE/ACT.
            wet_tiles = {}

            def expert_wet(e, mg):
                if (e, mg) not in wet_tiles:
                    wet = wep.tile([P, KT, 256], dt, tag="we",
                                   name=f"we_{e}_{mg}")
                    nc.sync.dma_start(
                        wet, we_d[e, mg].rearrange("p (k n) -> p k n", k=KT))
                    wet_tiles[(e, mg)] = wet
                return wet_tiles[(e, mg)]

            def emit_group(e, mg, mi, tb, tag="mm", bufs=3):
                wet = expert_wet(e, mg)
                ps = pp.tile([P, 512], F32, tag=tag, bufs=bufs, name="eps")
                for k in range(KT):
                    nc.tensor.matmul(ps, wet[:, k, mi * P:(mi + 1) * P],
                                     hT[:, k, tb * 512:(tb + 1) * 512],
                                     start=(k == 0), stop=(k == KT - 1))
                return ps

            pre_ps = {}
            for tb in range(TB):
                pre_ps[(0, 0, 0, tb)] = emit_group(0, 0, 0, tb, tag="hst",
                                                   bufs=2)
            if not include_be and MT >= 4:
                # 3 groups = all 3 "mm" banks parked; the 4th would deadlock
                # the in-order PE queue behind the not-yet-emitted broadcasts.
                for key in [(0, 0, 1, tb) for tb in range(TB)][:3 if TB >= 3
                                                              else TB] + \
                           [(0, 1, 0, tb) for tb in range(TB)][:3 - TB]:
                    pre_ps[key] = emit_group(*key)
            for tt in range(TT):
                trp = pp.tile([P, 4, P], F32, tag="tr", bufs=2, name="trp_ow")
                nc.tensor.matmul(trp[:E, 0, :], opw[:, tt, :], ident,
                                 is_transpose=True, start=True, stop=True)
                nc.scalar.copy(opwT[:, tt * P:(tt + 1) * P], trp[:E, 0, :])
            # all E*TB op_w broadcasts precomputed once -> no dependency
            # stalls at expert boundaries ("tr" banks: the "mm" banks may be
            # parked by the hoisted groups above)
            obsall = const.tile([P, E, TB, 512], dt, name="obsall")
            for e in range(E):
                for tb in range(TB):
                    bps = pp.tile([P, 512], F32, tag="tr", bufs=2, name="bps")
                    nc.tensor.matmul(bps, sel8[:, e * P:(e + 1) * P],
                                     opwT[:, tb * 512:(tb + 1) * 512],
                                     start=True, stop=True)
                    nc.scalar.copy(obsall[:, e, tb, :], bps)

            # ---- stage C: expert GEMMs in [H_out, T] orientation.
            # Stationary = We m-chunk, moving = resident hT at N=512.
            # op_w[t, e] is broadcast across partitions as
            # obs = sel8[:, e].T @ opwT (a K=8 matmul), and the weighted
            # combine accumulates straight into the arena (= combT [H, T]):
            #     combT[m, t] += obs[t] * psum[m, t]
            # fp32 accumulation arena; the LAST expert's combine add writes
            # the bf16 copy (arenaB) that stage E consumes as its moving
            # operand - the downcast costs no extra engine ops.
            arena = bigp.tile([P, KT, TT, P], F32, tag="B", name="arena")
            arenaR = arena.bitcast(F32R)
            arenaB = bigp.tile([P, KT, T], dt, tag="C", name="arenaB")

            if include_be:
                # init combT with the op_w-weighted bias term:
                #   combT[m*128+p, t] = sum_e op_w[t, e] * be[e, m*128+p]
                for m in range(MT):
                    for tb in range(TB):
                        bps = pp.tile([P, 512], F32, tag="mm", bufs=3, name="bps")
                        nc.tensor.matmul(bps, be_t[:, m * P:(m + 1) * P],
                                         opwT[:, tb * 512:(tb + 1) * 512],
                                         start=True, stop=True)
                        nc.scalar.copy(
                            arenaR[:, m, tb * 4:(tb + 1) * 4, :],
                            bps.rearrange("p (n c) -> p n c", c=P))

            for e in range(E):
                for mg in range(MT // 2):
                    for mi in range(2):
                        m = 2 * mg + mi
                        for tb in range(TB):
                            ps = pre_ps.pop((e, mg, mi, tb), None)
                            if ps is None:
                                ps = emit_group(e, mg, mi, tb)
                            wsl = arenaR[:, m, tb * 4:(tb + 1) * 4, :]
                            rsl = arena[:, m, tb * 4:(tb + 1) * 4, :]
                            bsl = arenaB[:, m, tb * 512:(tb + 1) * 512]
                            bsl3 = bsl.rearrange("p (n c) -> p n c", c=P)
                            ob3 = obsall[:, e, tb, :].rearrange(
                                "p (n c) -> p n c", c=P)
                            ps3 = ps.rearrange("p (n c) -> p n c", c=P)
                            if e == 0 and not include_be:
                                dst0 = bsl3 if E == 1 else wsl
                                nc.vector.tensor_tensor(dst0, ps3, ob3,
                                                        op=ALU.mult)
                            else:
                                tmp = scr.tile([P, 512], F32, tag="s",
                                               bufs=3, name="tmp")
                                tmp3 = tmp.rearrange("p (n c) -> p n c", c=P)
                                nc.vector.tensor_tensor(tmp3, ps3, ob3,
                                                        op=ALU.mult)
                                dst = bsl3 if e == E - 1 else wsl
                                nc.vector.tensor_tensor(dst, rsl, tmp3,
                                                        op=ALU.add)

            # ---- stage E: a2T = act(Wi1.T @ combT + bi1) ----
            a2T = bigp.tile([P, KT, T], dt, tag="A", name="a2T")
            for mg in range(MT // 2):
                w3 = wep.tile([P, KT, 256], dt, tag="we", name=f"wi1m_{mg}")
                nc.sync.dma_start(
                    w3, wi1_d[mg].rearrange("p (k n) -> p k n", k=KT))
                for mi in range(2):
                    m = 2 * mg + mi
                    for tb in range(TB):
                        ps = pp.tile([P, 512], F32, tag="mm", bufs=3, name="ps_g3")
                        for k in range(KT):
                            nc.tensor.matmul(ps, w3[:, k, mi * P:(mi + 1) * P],
                                             arenaB[:, k, tb * 512:(tb + 1) * 512],
                                             start=(k == 0), stop=(k == KT - 1))
                        nc.scalar.activation(a2T[:, m, tb * 512:(tb + 1) * 512],
                                             ps, act, bias=bi1_t[:, m:m + 1])

            # ---- stage F: outT = Wi2.T @ a2T + bi2, evicted straight to a
            #      small rotating buffer and DMA'd out (no transposes) ----
            for mg in range(MT // 2):
                w4 = wep.tile([P, KT, 256], dt, tag="we", name=f"wi2m_{mg}")
                nc.sync.dma_start(
                    w4, wi2_d[mg].rearrange("p (k n) -> p k n", k=KT))
                for mi in range(2):
                    m = 2 * mg + mi
                    for tb in range(TB):
                        ps = pp.tile([P, 512], F32, tag="mm", bufs=3, name="ps_g4")
                        for k in range(KT):
                            nc.tensor.matmul(ps, w4[:, k, mi * P:(mi + 1) * P],
                                             a2T[:, k, tb * 512:(tb + 1) * 512],
                                             start=(k == 0), stop=(k == KT - 1))
                        ot = osm.tile([P, 512], F32, tag="os", name="ot")
                        if apply_mask:
                            tmpo = scr.tile([P, 512], F32, tag="s", bufs=3,
                                            name="tmpo")
                            nc.scalar.activation(tmpo, ps, AF.Identity,
                                                 bias=bi2_t[:, m:m + 1])
                            nc.vector.tensor_tensor(
                                ot.rearrange("p (n c) -> p n c", c=P),
                                tmpo.rearrange("p (n c) -> p n c", c=P),
                                maskb[:, tb * 4:(tb + 1) * 4, :], op=ALU.mult)
                        else:
                            nc.scalar.activation(ot, ps, AF.Identity,
                                                 bias=bi2_t[:, m:m + 1])
                        nc.sync.dma_start(
                            out_d[m * P:(m + 1) * P,
                                  tb * 512:(tb + 1) * 512], ot)

    nc.compile()
    return nc


_CACHED = {}


def _get_nc(T, H, E, include_be, apply_mask):
    key = (T, H, E, include_be, apply_mask)
    if key not in _CACHED:
        _CACHED[key] = build_nc(T, H, E, act=AF.Gelu, include_be=include_be,
                                apply_mask=apply_mask)
    return _CACHED[key]


def _to_dt(x):
    if GEMM_DT == BF16:
        import ml_dtypes
        return np.ascontiguousarray(x.astype(ml_dtypes.bfloat16))
    return x


def _pack_w(w):
    """[H, H] weight -> [MG, P, KT*256] where [mg, p, k*256+n] =
    w[k*128+p, mg*256+n], so each (mg, p) line is DRAM-contiguous."""
    Hk, Hn = w.shape
    KT, MG = Hk // 128, Hn // 256
    return np.ascontiguousarray(
        w.reshape(KT, 128, MG, 256).transpose(2, 1, 0, 3).reshape(
            MG, 128, KT * 256))


def _pack_ht(hT):
    """[H, T] activation -> [TB, KG, P, 4*512] where
    [tb, kg, p, k*512+t] = hT[(kg*4+k)*128+p, tb*512+t]."""
    Hk, T = hT.shape
    KG, TB = Hk // 512, T // 512
    return np.ascontiguousarray(
        hT.reshape(KG, 4, 128, TB, 512).transpose(3, 0, 2, 1, 4).reshape(
            TB, KG, 128, 4 * 512))


def kernel(hidden_states, attention_mask, Wd1, bd1, Wd2, bd2, We, be, Wi1, bi1,
           Wi2, bi2, _trace=False):
    f32 = lambda x: np.ascontiguousarray(np.asarray(x, dtype=np.float32))
    h = f32(hidden_states)
    mask = f32(attention_mask)
    Wd1, bd1, Wd2, bd2 = f32(Wd1), f32(bd1), f32(Wd2), f32(bd2)
    We, be, Wi1, bi1, Wi2, bi2 = f32(We), f32(be), f32(Wi1), f32(bi1), f32(Wi2), f32(bi2)

    Bv, Sv, Hv = h.shape
    Ev = Wd2.shape[1]
    TOK = Bv * Sv
    T = TOK // N_CORES
    include_be = bool(np.any(be))
    apply_mask = not bool(np.all(mask == 1.0))

    nc = _get_nc(T, Hv, Ev, include_be, apply_mask)

    hTf = _to_dt(h.reshape(TOK, Hv).T)                 # [H, TOK]
    mf = mask.reshape(TOK)
    we_p = np.stack([_pack_w(w) for w in _to_dt(We)])
    weights = dict(wd1=_pack_w(_to_dt(Wd1)), bd1=bd1, wd2=_to_dt(Wd2),
                   bd2=bd2, we=we_p, be=be, wi1=_pack_w(_to_dt(Wi1)),
                   bi1=bi1, wi2=_pack_w(_to_dt(Wi2)), bi2=bi2)
    in_maps = []
    for c in range(N_CORES):
        m = dict(weights)
        m["ht"] = _pack_ht(hTf[:, c * T:(c + 1) * T])
        m["mask"] = np.ascontiguousarray(mf[c * T:(c + 1) * T])
        in_maps.append(m)

    # The first execution of a freshly-loaded NEFF occasionally trips a
    # transient NRT_EXEC_UNIT_UNRECOVERABLE on the axon worker; a retry after a
    # short pause has always succeeded, so tolerate a couple of those.
    last_exc = None
    for attempt in range(3):
        try:
            res = run_bass_kernel_spmd(nc, in_maps,
                                       core_ids=list(range(N_CORES)),
                                       trace=_trace)
            break
        except Exception as e:  # noqa: BLE001 - jax.errors.JaxRuntimeError
            last_exc = e
            if "UNAVAILABLE" not in str(e) and "unrecoverable" not in str(e):
                raise
            import time as _time
            _time.sleep(5 * (attempt + 1))
    else:
        raise last_exc
    out = np.concatenate(
        [np.asarray(res.results[c]["out"]).T for c in range(N_CORES)], axis=0)
    out = np.ascontiguousarray(out.reshape(Bv, Sv, Hv).astype(np.float32))
    if _trace:
        kernel._last_results = res
    return out
